# revision 1
# baseline (speedup 1.0000x reference)
"""Trainium2 kernel for nn_BernNet_47364899340878.

Math note (why the device kernel is just the MLP):
  The reference computes  out = sum_{j=0..K} c_j * relu(temp_j) * L^j (2I-L)^{K-j} h
  with c_j = C(K,j)/2^K and h = relu(x@W1+b1)@W2+b2.  The graded inputs pin
  temp = ones (spec fill "ones"), so relu(temp_j) = 1 for all j.  L and
  (2I - L) are commuting polynomials in the normalized adjacency, so the
  binomial theorem gives

      sum_j C(K,j) L^j (2I-L)^{K-j} = (L + 2I - L)^K = (2I)^K = 2^K I,

  i.e. the whole K=10 Bernstein propagation is exactly the identity map and
  out == h (verified numerically: |ref - h| <= ~5e-6 relative, pure fp32
  accumulation noise of the 20 gather/scatter hops).

Device kernel: h = relu(x@W1+b1)@W2+b2 and log_softmax(h), row-sharded over
8 NeuronCores (12500 rows each).  Host prep passes x transposed (so the
contraction dim lands on SBUF partitions without any on-device transpose),
pads K 500->512, and augments the weights:
  - W1 gains a 65th output column of zeros whose bias is 1.0, so the matmul
    itself produces a row of ones in h^T,
  - W2 gains a 65th input row equal to b2, so the ones row folds the second
    bias into the second matmul.
A non-ones temp (never the case for the graded inputs) falls back to a host
implementation of the propagation for correctness.

Perf notes (from NTFF traces; ~121us/core vs ~81us DMA roofline):
  - matmuls run as float32r (1 cyc/row when the moving dim >= 256, vs 4
    cyc/row for exact fp32 which lowers to two half-rate passes),
  - bias+relu is fused into one DVE tensor_scalar; Exp and Ln are pinned to
    their shared ACT table set so the whole kernel does one table load
    (each set switch costs ~1.3-2.7us),
  - log_softmax skips the max-subtraction: logits are bounded (|h| < ~6)
    so exp() cannot overflow, and log(sum(exp)) is exact enough in fp32,
  - outputs (raw+logp) ship as one DMA per block from the ACT HWDGE queue
    with a partition-contiguous DRAM layout (DMA cost scales with segment
    count; 160B segments were ~6x slower than 1280B),
  - inputs stream on the SP queue, one 1MB DMA per 500-row block.
"""

import numpy as np

_N = 100000
_FIN = 500
_HID = 64
_CLS = 40
_NCORES = 8
_RPC = _N // _NCORES  # 12500 rows per core
_KPAD = 512
_KC = 4  # contraction chunks of 128
_BLK = 500  # rows per block
_SUB = 125  # rows per mm2 subtile (4 per block)
_NSUB = 4

_CACHE = {}


def _build_bass(rpc, mm_r=True):
    """Build the per-core Bass program for `rpc` rows (rpc % 500 == 0)."""
    from contextlib import ExitStack

    import concourse.bacc as bacc
    import concourse.mybir as mybir
    import concourse.tile as tile

    fp32 = mybir.dt.float32
    # float32r: TF32-like PE mode — 4-byte storage, low 12 mantissa bits
    # dropped, 1 cycle/row (vs 4 for exact fp32, which runs as two half-rate
    # passes).  The BIR verifier requires fp32r matmul operands to be
    # *produced* as fp32r, so the SBUF tiles are declared fp32r (DRAM stays
    # fp32 for the PJRT interface; host pre-rounds the data) and the DVE
    # bias+relu writes its output directly as fp32r.
    mm_dt = mybir.dt.float32r if mm_r else mybir.dt.float32
    AF = mybir.ActivationFunctionType
    OP = mybir.AluOpType
    nblk = rpc // _BLK

    def asmm(ap):
        return ap.bitcast(mm_dt) if mm_r else ap

    # Bacc (not plain Bass): its compile() runs move_matmul_waits_to_ldweights
    # + generate_event_semaphores, which split excess on_wait entries to meet
    # TRN2's 1-wait-per-instruction constraint that walrus enforces.
    #
    # Table-set pinning: ACT function tables are loaded as named sets and a
    # set switch costs ~1.3-2.7us.  Exp and Ln both live in the
    # "natural_log_exp_and_others" set, but the default insertion pass picks
    # each function's first containing set, so an Exp/Ln mix reloads on every
    # switch.  Restricting Exp/Ln to their shared set (keeping every set's
    # positional id intact) makes the whole kernel need exactly one load.
    class _PinnedActBacc(bacc.Bacc):
        def insert_act_table_loads(self):
            import bass_rust as _bass_rust
            from concourse.hw_specs import get_activation_tables

            has_activation = any(
                isinstance(i, mybir.InstActivation)
                for b in self.main_func.blocks
                for i in b.instructions
            )
            if not has_activation:
                return
            shared = {AF.Exp, AF.Ln}
            tables = []
            for name, fns in get_activation_tables(self.m.arch).items():
                if name != "natural_log_exp_and_others":
                    fns = fns - shared
                tables.append((name, fns))
            _bass_rust.insert_act_table_loads(self, tables)

    nc = _PinnedActBacc()
    xt = nc.dram_tensor("xt", [_KPAD, rpc], fp32, kind="ExternalInput")
    w1 = nc.dram_tensor("w1", [_KPAD, _HID + 1], fp32, kind="ExternalInput")
    b1 = nc.dram_tensor("b1", [_HID + 1, 1], fp32, kind="ExternalInput")
    w2 = nc.dram_tensor("w2", [_HID + 1, _CLS], fp32, kind="ExternalInput")
    # output layout [blk, p, 2, si, c]: each partition's raw+logp data is one
    # contiguous 1280B run in DRAM, so a block's output DMA is 125 large
    # segments (a 160B-segment row-major layout was ~6x slower, segment-count
    # bound).  The host permutes (p, si) -> rows afterwards.
    both = nc.dram_tensor(
        "both", [rpc // _BLK, _SUB, 2, _NSUB, _CLS], fp32, kind="ExternalOutput"
    )

    xt_r = xt.rearrange("(kc p) (blk r) -> blk p kc r", p=128, r=_BLK)
    both_r = both.rearrange("blk p j si c -> blk p j si c")

    with tile.TileContext(nc) as tc, ExitStack() as ctx:
        const = ctx.enter_context(tc.tile_pool(name="const", bufs=1))
        xpool = ctx.enter_context(tc.tile_pool(name="xin", bufs=6))
        hpool = ctx.enter_context(tc.tile_pool(name="hrelu", bufs=2))
        epool = ctx.enter_context(tc.tile_pool(name="expv", bufs=3))
        opool = ctx.enter_context(tc.tile_pool(name="outs", bufs=3))
        spool = ctx.enter_context(tc.tile_pool(name="sums", bufs=3))
        lpool = ctx.enter_context(tc.tile_pool(name="lsub", bufs=3))
        pp1 = ctx.enter_context(tc.tile_pool(name="ps1", bufs=3, space="PSUM"))
        pp2 = ctx.enter_context(tc.tile_pool(name="ps2", bufs=3, space="PSUM"))

        # issue block-0's (big) input DMA first so the transfer overlaps the
        # small weight loads
        xt_first = xpool.tile([128, _KC, _BLK], mm_dt, tag="xt")
        nc.sync.dma_start(xt_first[:], asmm(xt_r[0]))


        w1_sb = const.tile([128, _KC, _HID + 1], mm_dt)
        nc.sync.dma_start(w1_sb[:], asmm(w1.rearrange("(kc p) m -> p kc m", p=128)))
        b1_sb = const.tile([_HID + 1, 1], fp32)
        nc.sync.dma_start(b1_sb[:], b1[:])
        w2_sb = const.tile([_HID + 1, _CLS], mm_dt)
        nc.sync.dma_start(w2_sb[:], asmm(w2[:]))

        def mm1_block(blk, xt_sb):
            # h^T = (W1p^T @ x^T) : [65, 500], accumulated over 4 K-chunks
            ht_ps = pp1.tile([_HID + 1, _BLK], fp32)
            for kc in range(_KC):
                nc.tensor.matmul(
                    ht_ps[:],
                    w1_sb[:, kc, :],
                    xt_sb[:, kc, :],
                    start=(kc == 0),
                    stop=(kc == _KC - 1),
                )
            return ht_ps

        def rest_block(blk, ht_ps):
            # fused bias+relu on DVE: max(ht + b1, 0); row 64 = max(0+1,0) = 1
            ht_relu = hpool.tile([_HID + 1, _BLK], mm_dt)
            nc.vector.tensor_scalar(
                out=ht_relu[:], in0=ht_ps[:], scalar1=b1_sb[:], scalar2=0.0,
                op0=OP.add, op1=OP.max,
            )

            # out = h_relu_aug^T.T @ W2_aug : 4 subtiles of 125 rows
            o_ps = pp2.tile([_SUB, _NSUB, _CLS], fp32)
            for si in range(_NSUB):
                nc.tensor.matmul(
                    o_ps[:, si, :],
                    ht_relu[:, si * _SUB : (si + 1) * _SUB],
                    w2_sb[:],
                )

            # raw logits + logp share one combined tile -> single output DMA
            cmb = opool.tile([_SUB, 2, _NSUB, _CLS], fp32)
            nc.vector.tensor_copy(cmb[:, 0], o_ps[:])

            # log_softmax without max-subtraction (logits bounded): Exp and
            # Ln share one ACT table set, so the whole chain stays in-loop.
            e_sb = epool.tile([_SUB, _NSUB, _CLS], fp32)
            nc.scalar.activation(e_sb[:], cmb[:, 0], AF.Exp)
            ssum = spool.tile([_SUB, _NSUB], fp32)
            nc.vector.tensor_reduce(
                out=ssum[:], in_=e_sb[:], op=OP.add, axis=mybir.AxisListType.X,
            )
            lse = lpool.tile([_SUB, _NSUB], fp32)
            nc.scalar.activation(lse[:], ssum[:], AF.Ln)
            nc.vector.tensor_sub(
                cmb[:, 1],
                cmb[:, 0],
                lse[:, :, None].broadcast_to([_SUB, _NSUB, _CLS]),
            )
            # one DMA for raw+logp, issued from the ACT HWDGE queue (SP is
            # saturated with input transfers; GpSimd SWDGE was ~2x slower)
            nc.scalar.dma_start(both_r[blk], cmb[:])

        # process blocks in pairs: 8 consecutive mm1 matmuls give the PE a
        # >4us uninterrupted burst, which is what the HAM activity monitor
        # needs to lift the PE clock out of its cold half-rate state
        for blk in range(0, nblk, 2):
            pair = []
            for b in (blk, blk + 1):
                if b >= nblk:
                    break
                if b == 0:
                    xt_sb = xt_first
                else:
                    xt_sb = xpool.tile([128, _KC, _BLK], mm_dt, tag="xt")
                    nc.sync.dma_start(xt_sb[:], asmm(xt_r[b]))
                pair.append((b, xt_sb))
            hts = [(b, mm1_block(b, xt_sb)) for b, xt_sb in pair]
            for b, ht_ps in hts:
                rest_block(b, ht_ps)

    nc.finalize()
    return nc


def _get_bass(rpc):
    key = ("nc", rpc)
    if key not in _CACHE:
        _CACHE[key] = _build_bass(rpc)
    return _CACHE[key]


def _round_fp32r(a):
    """Round fp32 to float32r's grid (low 12 mantissa bits dropped, RNE),
    matching the PE's reduced-precision fp32 mode."""
    b = np.ascontiguousarray(a, np.float32).view(np.uint32)
    r = b + 0x7FF + ((b >> 12) & 1)
    return (r & np.uint32(0xFFFFF000)).view(np.float32)


def _host_prep(x, W1, b1, W2, b2):
    x = np.asarray(x, np.float32)
    xt = np.zeros((_KPAD, _N), np.float32)
    xt[:_FIN] = _round_fp32r(x).T
    w1p = np.zeros((_KPAD, _HID + 1), np.float32)
    w1p[:_FIN, :_HID] = _round_fp32r(np.asarray(W1, np.float32))
    b1a = np.zeros((_HID + 1, 1), np.float32)
    b1a[:_HID, 0] = np.asarray(b1, np.float32)
    b1a[_HID, 0] = 1.0
    w2a = np.zeros((_HID + 1, _CLS), np.float32)
    w2a[:_HID] = _round_fp32r(np.asarray(W2, np.float32))
    w2a[_HID] = _round_fp32r(np.asarray(b2, np.float32))
    return xt, w1p, b1a, w2a


def _bern_prop_host(h, edge_index, theta):
    """Fallback: full Bernstein propagation on host (only if temp != ones)."""
    from math import comb

    n = h.shape[0]
    src = np.asarray(edge_index[0], np.int64)
    dst = np.asarray(edge_index[1], np.int64)
    deg = np.bincount(src, minlength=n).astype(np.float32)
    dis = np.where(deg > 0, 1.0 / np.sqrt(np.maximum(deg, 1.0)), 0.0).astype(
        np.float32
    )

    def anorm(v):
        msg = v[src] * dis[src][:, None]
        out = np.zeros_like(v)
        np.add.at(out, dst, msg)
        return out * dis[:, None]

    K = len(theta) - 1
    tmp = [h]
    for _ in range(K):
        t = tmp[-1]
        tmp.append(t + anorm(t))
    c = np.array([comb(K, j) / 2.0**K for j in range(K + 1)], np.float32)
    acc = np.zeros_like(h)
    for j in range(K, 0, -1):
        s = acc + c[j] * theta[j] * tmp[K - j]
        acc = s - anorm(s)
    return c[0] * theta[0] * tmp[K] + acc


def kernel(x, edge_index, W1, b1, W2, b2, temp):
    from concourse.bass_utils import run_bass_kernel_spmd

    xt, w1p, b1a, w2a = _host_prep(x, W1, b1, W2, b2)
    nc = _get_bass(_RPC)
    in_maps = []
    for c in range(_NCORES):
        shard = np.ascontiguousarray(xt[:, c * _RPC : (c + 1) * _RPC])
        in_maps.append({"xt": shard, "w1": w1p, "b1": b1a, "w2": w2a})
    res = run_bass_kernel_spmd(nc, in_maps, core_ids=list(range(_NCORES)))
    # both: [nblk, p(125), 2, si(4), c] -> rows ordered (blk, si, p)
    def unshard(c, j):
        a = res.results[c]["both"][:, :, j]            # [nblk, p, si, cls]
        return a.transpose(0, 2, 1, 3).reshape(_RPC, _CLS)

    out = np.concatenate([unshard(c, 0) for c in range(_NCORES)])
    lp = np.concatenate([unshard(c, 1) for c in range(_NCORES)])

    theta = np.maximum(np.asarray(temp, np.float32), 0.0)
    if not np.allclose(theta, 1.0):
        # General-temp path: device computed h; propagate on host, then
        # recompute log_softmax.
        out = _bern_prop_host(out.astype(np.float32), edge_index, theta)
        m = out.max(axis=1, keepdims=True)
        lp = out - (np.log(np.exp(out - m).sum(axis=1, keepdims=True)) + m)
        lp = lp.astype(np.float32)

    return lp, out

